# revision 7
# baseline (speedup 1.0000x reference)
"""Trainium2 Bass kernel for nn_Bernstein (gnn_message_passing).

Math: the reference Bernstein-polynomial GNN collapses algebraically to
    out[b] = sum_{k=0..5} (L^k x_b) @ U_k
where U_k are 64x64 folded weight matrices (including the reference's
faithful stale-x3 quirk: stack_5 = theta_5 * stack_4).

Sharding: batch b -> core b (8 cores, zero collectives).  Each core runs
5 SpMMs (Krylov chain).  The SpMM gathers PAIRS of rows (512B) from an
HBM-resident fp16 table [M/2, 128] via gpsimd.dma_gather (int16 pair
indices), selects the right parity via doubled per-edge values
(vals * parity mask) in a DVE broadcast multiply, tree-reduces over the
9 neighbors, and stores the next Krylov table.  The 64x64 projection is
accumulated on the PE into a transposed fp16 accumulator (U doubled
block-diagonally to keep row parity separated), finalized with PE
transposes interleaved into the last SpMM.
"""

import numpy as np
from math import comb

# ---------------- problem constants (hardcoded per contract) ----------------
B = 8
M = 49152
F = 64          # feature dim (both in and out)
KPOW = 5        # polynomial degree
N_CORES = 8

# kernel tiling
T = 8                  # 128-row tiles per supertile
ST = 128 * T           # rows per supertile (1024)
NS = M // ST           # supertiles (48)
PAIRS = M // 2         # paired-row count for fp16 tables (24576)
CHP = 512              # pairs per projection chunk (=1024 rows)
NCH = PAIRS // CHP     # projection chunks (48)
CH_PER_ST = ST // (2 * CHP)  # proj chunks per supertile (1)


def _theta_coeffs(deg):
    """c[i, k]: coefficient of L^k in theta_i * (2I - L)^(deg-i) L^i,
    with the reference's stale-x3 quirk for i == deg."""
    c = np.zeros((deg + 1, deg + 1), dtype=np.float64)
    for i in range(deg):
        theta = comb(deg, i) / 2 ** deg
        for j in range(deg - i + 1):
            c[i, i + j] += theta * comb(deg - i, j) * (2.0 ** (deg - i - j)) * ((-1.0) ** j)
    c[deg, :] = (comb(deg, deg) / 2 ** deg) * c[deg - 1, :]
    return c


def _host_prep(L_vals, kernel, L_rows, L_cols):
    """Pure index/layout preprocessing + weight folding (no model FLOPs on data)."""
    nnz = L_rows.shape[0]
    cnt = np.bincount(L_rows, minlength=M)
    deg = int(cnt.max())
    if deg * M == nnz and np.all(cnt == deg):
        cols_d = L_cols.reshape(M, deg).astype(np.int64)
        vals_d = L_vals.reshape(M, deg).astype(np.float32)
    else:
        # general sorted-COO: pad rows to fixed capacity (vals 0 -> no effect)
        cols_d = np.zeros((M, deg), dtype=np.int64)
        vals_d = np.zeros((M, deg), dtype=np.float32)
        starts = np.concatenate([[0], np.cumsum(cnt)[:-1]])
        pos = np.arange(nnz) - starts[L_rows]
        cols_d[L_rows, pos] = L_cols
        vals_d[L_rows, pos] = L_vals

    # row mapping m(s, p, t) = s*ST + p*T + t ; gather slot (d*T + t), partition p
    m_idx = (np.arange(NS)[:, None, None] * ST
             + np.arange(128)[None, :, None] * T
             + np.arange(T)[None, None, :])           # [NS, 128, T]
    cols_spt = cols_d[m_idx]                          # [NS, 128, T, deg]
    vals_spt = vals_d[m_idx]                          # [NS, 128, T, deg]

    # gather order: j = (d*T + t)*128 + p  ->  idx_flat[s][j] = col >> 1
    # idx_flat as [NS, d, t, p]
    idx_dtp = (cols_spt >> 1).transpose(0, 3, 2, 1)   # [NS, deg, T, 128]
    idx_flat = idx_dtp.reshape(NS, deg * T * 128)
    # int16 wrapped-by-16 layout replicated across the 8 core groups:
    # wrapped[s, p, i] = idx_flat[s, i*16 + (p % 16)]
    n_i = deg * T * 128 // 16
    wrapped = idx_flat.reshape(NS, n_i, 16)           # [NS, i, p%16]
    wrapped = wrapped.transpose(0, 2, 1)              # [NS, 16, i]
    wrapped = np.tile(wrapped, (1, 8, 1)).astype(np.int16)  # [NS, 128, i]

    # doubled vals with parity select: vals2[s, p, d*T+t, q] = val * (par == q)
    par = (cols_spt & 1)                              # [NS, 128, T, deg]
    v2 = np.zeros((NS, 128, T, deg, 2), dtype=np.float32)
    np.put_along_axis(v2, par[..., None], vals_spt[..., None], axis=-1)
    vals2 = v2.transpose(0, 1, 3, 2, 4).reshape(NS, 128, deg * T, 2)
    vals2 = np.ascontiguousarray(vals2).astype(np.float16)

    # folded projection weights, doubled block-diagonal for the paired layout
    c = _theta_coeffs(KPOW)
    Wr = kernel.astype(np.float64).reshape(F, KPOW + 1, F)
    U = np.einsum('ik,fio->kfo', c, Wr)               # [6, F, F]
    U2 = np.zeros((KPOW + 1, 2 * F, 2 * F), dtype=np.float16)
    for k in range(KPOW + 1):
        U2[k, :F, :F] = U[k]
        U2[k, F:, F:] = U[k]
    return wrapped, vals2, U2, deg


def build_program(deg, num_devices=N_CORES):
    import concourse.bacc as bacc
    import concourse.tile as tile
    import concourse.mybir as mybir
    from concourse import bass
    from concourse.masks import make_identity

    fp16 = mybir.dt.float16
    fp32 = mybir.dt.float32
    i16 = mybir.dt.int16

    NSLOT = deg * T                 # gather slots per supertile (72)
    NIDX = NSLOT * 128              # gathers per supertile (9216)
    NWRAP = NIDX // 16              # wrapped idx free dim (576)

    nc = bacc.Bacc("TRN2", target_bir_lowering=False, debug=False,
                   num_devices=num_devices, num_swdge_queues=4)

    # ---- I/O ----
    x_in = nc.dram_tensor("x_in", [M, F], fp32, kind="ExternalInput").ap()
    idx_in = nc.dram_tensor("idx16", [NS, 128, NWRAP], i16, kind="ExternalInput").ap()
    vals_in = nc.dram_tensor("vals2", [NS, 128, NSLOT, 2], fp16, kind="ExternalInput").ap()
    u2_in = nc.dram_tensor("u2", [KPOW + 1, 2 * F, 2 * F], fp16, kind="ExternalInput").ap()
    out_y = nc.dram_tensor("out_y", [M, F], fp32, kind="ExternalOutput").ap()

    # ---- DRAM scratch: ping-pong fp16 tables in paired layout [PAIRS, 128]
    tabs = [nc.dram_tensor(f"tab{i}", [PAIRS, 2 * F], fp16, kind="Internal").ap()
            for i in range(2)]

    with tile.TileContext(nc) as tc:
        with tc.tile_pool(name="persist", bufs=1) as pp, \
             tc.tile_pool(name="work", bufs=3) as wp, \
             tc.tile_pool(name="proj", bufs=3) as jp, \
             tc.tile_pool(name="psum", bufs=2, space="PSUM") as sp:

            # ---- persistent SBUF ----
            vals_sb = pp.tile([128, NS, NSLOT, 2], fp16)
            u2_sb = pp.tile([2 * F, KPOW + 1, 2 * F], fp16)
            ident = pp.tile([128, 128], fp16)
            acc = pp.tile([128, PAIRS], fp16)   # partition=(parity, fo), free=pair

            nc.sync.dma_start(out=vals_sb[:], in_=vals_in.transpose([1, 0, 2, 3]))
            nc.sync.dma_start(out=u2_sb[:], in_=u2_in.transpose([1, 0, 2]))
            make_identity(nc, ident[:])

            # V_0: cast input to fp16 paired table (DRAM->DRAM cast via SWDGE)
            nc.gpsimd.dma_start(
                out=tabs[0][:],
                in_=x_in.rearrange("(a b) c -> a (b c)", b=2),
            )

            def proj_chunk(k, src_tab, c):
                """acc[:, c*CHP:(c+1)*CHP] (+)= U2_k.T @ table^T chunk."""
                vt = jp.tile([128, CHP], fp16, tag="vt")
                nc.sync.dma_start(
                    out=vt[:],
                    in_=src_tab[c * CHP:(c + 1) * CHP, :],
                    transpose=True,
                )
                ps = sp.tile([128, CHP], fp32, tag="mm")
                nc.tensor.matmul(
                    out=ps[:], lhsT=u2_sb[:, k, :], rhs=vt[:],
                    start=True, stop=True,
                )
                dst = acc[:, c * CHP:(c + 1) * CHP]
                if k == 0:
                    nc.any.tensor_copy(out=dst, in_=ps[:])
                else:
                    nc.any.tensor_add(out=dst, in0=dst, in1=ps[:])

            def finalize_chunk(c):
                """Transpose acc chunk back to [m, f] fp32 and store."""
                for ii in range(CHP // 64):  # 8 m-tiles of 128 rows per chunk
                    i = c * (CHP // 64) + ii
                    pt = sp.tile([64, 128], fp16, tag="tr")
                    nc.tensor.transpose(
                        out=pt[:], in_=acc[:, i * 64:(i + 1) * 64], identity=ident[:],
                    )
                    st = jp.tile([64, 128], fp32, tag="st")
                    nc.any.tensor_copy(out=st[:], in_=pt[:])
                    nc.sync.dma_start(
                        out=out_y[i * 128:(i + 1) * 128, :]
                        .rearrange("(j q) f -> j (q f)", j=64),
                        in_=st[:],
                    )

            for k in range(1, KPOW + 1):
                src, dst = tabs[(k - 1) % 2], tabs[k % 2]
                for s in range(NS):
                    # load wrapped int16 pair-indices for this supertile
                    osb = wp.tile([128, NWRAP], i16, tag="osb")
                    nc.sync.dma_start(out=osb[:], in_=idx_in[s])
                    # gather pair-rows: G[p, c, :] = src[idx[c*128+p], :]
                    # Split into <=1024-idx chunks (65 descs/ring, well under
                    # the 128-entry SWDGE ring) across the 4 SWDGE queues.
                    G = wp.tile([128, NSLOT, 2, F], fp16, tag="G")
                    GCH = T  # slots per gather chunk (1024 idxs)
                    for ci, a in enumerate(range(0, NSLOT, GCH)):
                        b = min(a + GCH, NSLOT)
                        nidx = (b - a) * 128
                        nc.gpsimd.dma_gather(
                            G[:, a:b, :, :].rearrange("p s q f -> p s (q f)"),
                            src[:], osb[:, a * 8:b * 8],
                            num_idxs=nidx, num_idxs_reg=nidx,
                            elem_size=2 * F,
                        )
                    # scale by parity-selected per-edge values (broadcast on f)
                    nc.vector.tensor_tensor(
                        out=G[:], in0=G[:],
                        in1=vals_sb[:, s, :, :].unsqueeze(-1).to_broadcast(
                            [128, NSLOT, 2, F]),
                        op=mybir.AluOpType.mult,
                    )
                    # fold the pair halves, then tree-reduce over d (in place)
                    lo = G[:, :, 0, :]
                    nc.any.tensor_add(out=lo, in0=lo, in1=G[:, :, 1, :])
                    d = deg
                    while d > 1:
                        h = d // 2
                        nc.any.tensor_add(
                            out=G[:, 0:h * T, 0, :], in0=G[:, 0:h * T, 0, :],
                            in1=G[:, h * T:2 * h * T, 0, :])
                        if d % 2:
                            nc.any.tensor_add(
                                out=G[:, 0:T, 0, :], in0=G[:, 0:T, 0, :],
                                in1=G[:, (d - 1) * T:d * T, 0, :])
                        d = h
                    # store V_k supertile (paired layout, contiguous)
                    nc.sync.dma_start(
                        out=dst[s * (ST // 2):(s + 1) * (ST // 2), :]
                        .rearrange("(p q) c -> p (q c)", p=128),
                        in_=G[:, 0:T, 0, :],
                    )
                    # interleave projection of V_{k-1} (fully available)
                    for cc in range(CH_PER_ST):
                        proj_chunk(k - 1, src, s * CH_PER_ST + cc)
                    if k == KPOW:
                        # V_5 projection + output finalize, per freshly stored chunk
                        for cc in range(CH_PER_ST):
                            c = s * CH_PER_ST + cc
                            proj_chunk(KPOW, dst, c)
                            finalize_chunk(c)

    nc.compile()
    return nc


_cache = {}


def _get_program(deg):
    if deg not in _cache:
        _cache[deg] = build_program(deg)
    return _cache[deg]


def kernel(input_tensor, L_vals, kernel, L_rows, L_cols):
    from concourse import bass_utils

    idx16, vals2, U2, deg = _host_prep(
        np.asarray(L_vals), np.asarray(kernel),
        np.asarray(L_rows), np.asarray(L_cols))
    nc = _get_program(deg)

    inp = np.asarray(input_tensor)
    in_maps = []
    for b in range(N_CORES):
        in_maps.append({
            "x_in": np.ascontiguousarray(inp[b]).astype(np.float32),
            "idx16": idx16,
            "vals2": vals2,
            "u2": U2,
        })
    res = bass_utils.run_bass_kernel_spmd(nc, in_maps, core_ids=list(range(N_CORES)))
    out = np.stack([res.results[b]["out_y"] for b in range(N_CORES)], axis=0)
    return out.astype(np.float32)


# revision 9
# speedup vs baseline: 34.3337x; 34.3337x over previous
"""Trainium2 Bass kernel for nn_Bernstein (gnn_message_passing).

Math: the reference Bernstein-polynomial GNN collapses algebraically to
    out[b] = sum_{k=0..5} (L^k x_b) @ U_k
where U_k are 64x64 folded weight matrices (including the reference's
faithful stale-x3 quirk: stack_5 = theta_5 * stack_4).

Sharding: batch b -> core b (8 cores, zero collectives).  Each core runs
5 SpMMs (Krylov chain).  The SpMM gathers PAIRS of rows (512B) from an
HBM-resident fp16 table [M/2, 128] via gpsimd.dma_gather (int16 pair
indices), selects the right parity via doubled per-edge values
(vals * parity mask) in a DVE broadcast multiply, tree-reduces over the
9 neighbors, and stores the next Krylov table.  The 64x64 projection is
accumulated on the PE into a transposed fp16 accumulator (U doubled
block-diagonally to keep row parity separated), finalized with PE
transposes interleaved into the last SpMM.
"""

import numpy as np
from math import comb

# ---------------- problem constants (hardcoded per contract) ----------------
B = 8
M = 49152
F = 64          # feature dim (both in and out)
KPOW = 5        # polynomial degree
N_CORES = 8

# kernel tiling
T = 8                  # 128-row tiles per supertile
ST = 128 * T           # rows per supertile (1024)
NS = M // ST           # supertiles (48)
PAIRS = M // 2         # paired-row count for fp16 tables (24576)
CHP = 512              # pairs per projection chunk (=1024 rows)
NCH = PAIRS // CHP     # projection chunks (48)
CH_PER_ST = ST // (2 * CHP)  # proj chunks per supertile (1)


def _theta_coeffs(deg):
    """c[i, k]: coefficient of L^k in theta_i * (2I - L)^(deg-i) L^i,
    with the reference's stale-x3 quirk for i == deg."""
    c = np.zeros((deg + 1, deg + 1), dtype=np.float64)
    for i in range(deg):
        theta = comb(deg, i) / 2 ** deg
        for j in range(deg - i + 1):
            c[i, i + j] += theta * comb(deg - i, j) * (2.0 ** (deg - i - j)) * ((-1.0) ** j)
    c[deg, :] = (comb(deg, deg) / 2 ** deg) * c[deg - 1, :]
    return c


def _host_prep(L_vals, kernel, L_rows, L_cols):
    """Pure index/layout preprocessing + weight folding (no model FLOPs on data)."""
    nnz = L_rows.shape[0]
    cnt = np.bincount(L_rows, minlength=M)
    deg = int(cnt.max())
    if deg * M == nnz and np.all(cnt == deg):
        cols_d = L_cols.reshape(M, deg).astype(np.int64)
        vals_d = L_vals.reshape(M, deg).astype(np.float32)
    else:
        # general sorted-COO: pad rows to fixed capacity (vals 0 -> no effect)
        cols_d = np.zeros((M, deg), dtype=np.int64)
        vals_d = np.zeros((M, deg), dtype=np.float32)
        starts = np.concatenate([[0], np.cumsum(cnt)[:-1]])
        pos = np.arange(nnz) - starts[L_rows]
        cols_d[L_rows, pos] = L_cols
        vals_d[L_rows, pos] = L_vals

    # row mapping m(s, p, t) = s*ST + p*T + t ; gather slot (d*T + t), partition p
    m_idx = (np.arange(NS)[:, None, None] * ST
             + np.arange(128)[None, :, None] * T
             + np.arange(T)[None, None, :])           # [NS, 128, T]
    cols_spt = cols_d[m_idx]                          # [NS, 128, T, deg]
    vals_spt = vals_d[m_idx]                          # [NS, 128, T, deg]

    # gather order: j = (d*T + t)*128 + p  ->  idx_flat[s][j] = col >> 1
    # idx_flat as [NS, d, t, p]
    idx_dtp = (cols_spt >> 1).transpose(0, 3, 2, 1)   # [NS, deg, T, 128]
    idx_flat = idx_dtp.reshape(NS, deg * T * 128)
    # int16 wrapped-by-16 layout replicated across the 8 core groups:
    # wrapped[s, p, i] = idx_flat[s, i*16 + (p % 16)]
    n_i = deg * T * 128 // 16
    wrapped = idx_flat.reshape(NS, n_i, 16)           # [NS, i, p%16]
    wrapped = wrapped.transpose(0, 2, 1)              # [NS, 16, i]
    wrapped = np.tile(wrapped, (1, 8, 1)).astype(np.int16)  # [NS, 128, i]

    # doubled vals with parity select: vals2[s, p, d*T+t, q] = val * (par == q)
    par = (cols_spt & 1)                              # [NS, 128, T, deg]
    v2 = np.zeros((NS, 128, T, deg, 2), dtype=np.float32)
    np.put_along_axis(v2, par[..., None], vals_spt[..., None], axis=-1)
    vals2 = v2.transpose(0, 1, 3, 2, 4).reshape(NS, 128, deg * T, 2)
    vals2 = np.ascontiguousarray(vals2).astype(np.float16)

    # folded projection weights, doubled block-diagonal for the paired layout
    c = _theta_coeffs(KPOW)
    Wr = kernel.astype(np.float64).reshape(F, KPOW + 1, F)
    U = np.einsum('ik,fio->kfo', c, Wr)               # [6, F, F]
    U2 = np.zeros((KPOW + 1, 2 * F, 2 * F), dtype=np.float16)
    for k in range(KPOW + 1):
        U2[k, :F, :F] = U[k]
        U2[k, F:, F:] = U[k]
    return wrapped, vals2, U2, deg


def build_program(deg, num_devices=N_CORES, repeats=1):
    import concourse.bacc as bacc
    import concourse.tile as tile
    import concourse.mybir as mybir
    from concourse import bass
    from concourse.masks import make_identity

    fp16 = mybir.dt.float16
    fp32 = mybir.dt.float32
    i16 = mybir.dt.int16

    NSLOT = deg * T                 # gather slots per supertile (72)
    NIDX = NSLOT * 128              # gathers per supertile (9216)
    NWRAP = NIDX // 16              # wrapped idx free dim (576)

    nc = bacc.Bacc("TRN2", target_bir_lowering=False, debug=False,
                   num_devices=num_devices, num_swdge_queues=4)

    # ---- I/O ----
    x_in = nc.dram_tensor("x_in", [M, F], fp32, kind="ExternalInput").ap()
    idx_in = nc.dram_tensor("idx16", [NS, 128, NWRAP], i16, kind="ExternalInput").ap()
    vals_in = nc.dram_tensor("vals2", [NS, 128, NSLOT, 2], fp16, kind="ExternalInput").ap()
    u2_in = nc.dram_tensor("u2", [KPOW + 1, 2 * F, 2 * F], fp16, kind="ExternalInput").ap()
    out_y = nc.dram_tensor("out_y", [M, F], fp32, kind="ExternalOutput").ap()

    # ---- DRAM scratch: ping-pong fp16 tables in paired layout [PAIRS, 128]
    tabs = [nc.dram_tensor(f"tab{i}", [PAIRS, 2 * F], fp16, kind="Internal").ap()
            for i in range(2)]

    with tile.TileContext(nc) as tc:
        with tc.tile_pool(name="persist", bufs=1) as pp, \
             tc.tile_pool(name="work", bufs=3) as wp, \
             tc.tile_pool(name="proj", bufs=3) as jp, \
             tc.tile_pool(name="psum", bufs=2, space="PSUM") as sp:

            # ---- persistent SBUF ----
            vals_sb = pp.tile([128, NS, NSLOT, 2], fp16)
            u2_sb = pp.tile([2 * F, KPOW + 1, 2 * F], fp16)
            ident = pp.tile([128, 128], fp16)
            acc = pp.tile([128, PAIRS], fp16)   # partition=(parity, fo), free=pair

            nc.sync.dma_start(out=vals_sb[:], in_=vals_in.transpose([1, 0, 2, 3]))
            nc.sync.dma_start(out=u2_sb[:], in_=u2_in.transpose([1, 0, 2]))
            make_identity(nc, ident[:])

            # V_0: cast input to fp16 paired table (DRAM->DRAM cast via SWDGE)
            nc.gpsimd.dma_start(
                out=tabs[0][:],
                in_=x_in.rearrange("(a b) c -> a (b c)", b=2),
            )

            def proj_chunk(k, src_tab, c):
                """acc[:, c*CHP:(c+1)*CHP] (+)= U2_k.T @ table^T chunk."""
                vt = jp.tile([128, CHP], fp16, tag="vt")
                nc.sync.dma_start(
                    out=vt[:],
                    in_=src_tab[c * CHP:(c + 1) * CHP, :],
                    transpose=True,
                )
                ps = sp.tile([128, CHP], fp32, tag="mm")
                nc.tensor.matmul(
                    out=ps[:], lhsT=u2_sb[:, k, :], rhs=vt[:],
                    start=True, stop=True,
                )
                dst = acc[:, c * CHP:(c + 1) * CHP]
                if k == 0:
                    nc.any.tensor_copy(out=dst, in_=ps[:])
                else:
                    nc.any.tensor_add(out=dst, in0=dst, in1=ps[:])

            def finalize_chunk(c):
                """Transpose acc chunk back to [m, f] fp32 and store."""
                for ii in range(CHP // 64):  # 8 m-tiles of 128 rows per chunk
                    i = c * (CHP // 64) + ii
                    pt = sp.tile([64, 128], fp16, tag="tr")
                    nc.tensor.transpose(
                        out=pt[:], in_=acc[:, i * 64:(i + 1) * 64], identity=ident[:],
                    )
                    st = jp.tile([64, 128], fp32, tag="st")
                    nc.any.tensor_copy(out=st[:], in_=pt[:])
                    nc.sync.dma_start(
                        out=out_y[i * 128:(i + 1) * 128, :]
                        .rearrange("(j q) f -> j (q f)", j=64),
                        in_=st[:],
                    )

            for k in [kk for _ in range(repeats) for kk in range(1, KPOW + 1)]:
                src, dst = tabs[(k - 1) % 2], tabs[k % 2]
                for s in range(NS):
                    # load wrapped int16 pair-indices for this supertile
                    osb = wp.tile([128, NWRAP], i16, tag="osb")
                    nc.sync.dma_start(out=osb[:], in_=idx_in[s])
                    # gather pair-rows: G[p, c, :] = src[idx[c*128+p], :]
                    # Split into <=1024-idx chunks (65 descs/ring, well under
                    # the 128-entry SWDGE ring) across the 4 SWDGE queues.
                    G = wp.tile([128, NSLOT, 2, F], fp16, tag="G")
                    GCH = T  # slots per gather chunk (1024 idxs)
                    for ci, a in enumerate(range(0, NSLOT, GCH)):
                        b = min(a + GCH, NSLOT)
                        nidx = (b - a) * 128
                        nc.gpsimd.dma_gather(
                            G[:, a:b, :, :].rearrange("p s q f -> p s (q f)"),
                            src[:], osb[:, a * 8:b * 8],
                            num_idxs=nidx, num_idxs_reg=nidx,
                            elem_size=2 * F,
                        )
                    # scale by parity-selected per-edge values (broadcast on f)
                    nc.vector.tensor_tensor(
                        out=G[:], in0=G[:],
                        in1=vals_sb[:, s, :, :].unsqueeze(-1).to_broadcast(
                            [128, NSLOT, 2, F]),
                        op=mybir.AluOpType.mult,
                    )
                    # fold the pair halves, then tree-reduce over d (in place)
                    lo = G[:, :, 0, :]
                    nc.any.tensor_add(out=lo, in0=lo, in1=G[:, :, 1, :])
                    d = deg
                    while d > 1:
                        h = d // 2
                        nc.any.tensor_add(
                            out=G[:, 0:h * T, 0, :], in0=G[:, 0:h * T, 0, :],
                            in1=G[:, h * T:2 * h * T, 0, :])
                        if d % 2:
                            nc.any.tensor_add(
                                out=G[:, 0:T, 0, :], in0=G[:, 0:T, 0, :],
                                in1=G[:, (d - 1) * T:d * T, 0, :])
                        d = h
                    # store V_k supertile (paired layout, contiguous)
                    nc.sync.dma_start(
                        out=dst[s * (ST // 2):(s + 1) * (ST // 2), :]
                        .rearrange("(p q) c -> p (q c)", p=128),
                        in_=G[:, 0:T, 0, :],
                    )
                    # interleave projection of V_{k-1} (fully available)
                    for cc in range(CH_PER_ST):
                        proj_chunk(k - 1, src, s * CH_PER_ST + cc)
                    if k == KPOW:
                        # V_5 projection + output finalize, per freshly stored chunk
                        for cc in range(CH_PER_ST):
                            c = s * CH_PER_ST + cc
                            proj_chunk(KPOW, dst, c)
                            finalize_chunk(c)

    nc.compile()
    return nc


_cache = {}


def _get_program(deg):
    if deg not in _cache:
        _cache[deg] = build_program(deg)
    return _cache[deg]


def kernel(input_tensor, L_vals, kernel, L_rows, L_cols):
    from concourse import bass_utils

    idx16, vals2, U2, deg = _host_prep(
        np.asarray(L_vals), np.asarray(kernel),
        np.asarray(L_rows), np.asarray(L_cols))
    nc = _get_program(deg)

    inp = np.asarray(input_tensor)
    in_maps = []
    for b in range(N_CORES):
        in_maps.append({
            "x_in": np.ascontiguousarray(inp[b]).astype(np.float32),
            "idx16": idx16,
            "vals2": vals2,
            "u2": U2,
        })
    res = bass_utils.run_bass_kernel_spmd(nc, in_maps, core_ids=list(range(N_CORES)))
    out = np.stack([res.results[b]["out_y"] for b in range(N_CORES)], axis=0)
    return out.astype(np.float32)
